# revision 1
# baseline (speedup 1.0000x reference)
"""BinaryLinear kernel for Trainium2, data-parallel over 8 NeuronCores.

Computes y = x @ (sign(W) * scale).T + b where
  sign(w) = +1 if w >= 0 else -1
  scale_o = max(mean_i |W[o,i]|, 1e-6)           (per output row)

Strategy
--------
- Shard batch (32768) across 8 cores -> 4096 rows/core; replicate weights.
- sign(W) and scale are computed on the HOST (scale from full-precision W,
  so that error source is gone entirely); the device only does matmuls and
  a fused scale*psum+bias epilogue.
- Mixed precision split of the 1024-long contraction, chosen so the
  measured max-rel error (1.79e-2) stays under the 2e-2 gate:
    k in [0,512):   x in fp8 e4m3, sign in fp8, matmul in DoubleRow perf
                    mode (two fp8 weights per PE cell -> K=256 per MM,
                    measured ~2x bf16 contraction rate)
    k in [512,1024): x in bf16, sign in fp8 (exact +-1; mixed-dtype
                    lhsT fp8 x rhs bf16 runs at the full bf16 rate and
                    halves the sign-matrix DMA), regular matmuls
  Per (batch-block, out-block) PSUM group: 2 DoubleRow MMs + 4 bf16 MMs
  accumulate f32 into one PSUM bank (measured 1.31us/group vs 1.73 all-
  bf16; 6 x 512-column streams is the accuracy-constrained PE floor).
- Skewed wave schedule: MM(unit u, out-block c) issues at wave u+c, so
  the 8 PSUM banks finish staggered ~1 wave apart instead of all in the
  final sweep.  Epilogues then never queue up and bank recycling never
  stalls the PE.  Block 0 instead runs its two DoubleRow sweeps first:
  the fp8 inputs (1MB) land well before the bf16 ones, giving the PE a
  ~4us runway that covers the remaining head DMA time.
- Head DMAs fan out over four engine queues (sync/scalar/vector/gpsimd)
  ordered so first-needed tiles lead each queue; later-stage x loads are
  queued behind them (per-queue FIFO) so they can't steal bandwidth from
  the critical head.  PE warm-up matmuls with no input deps cover the
  HAM clock-gate window (~3.4us) during the head so the real stream runs
  at 2.4 GHz from the start.
- Epilogues alternate DVE / ACT per out-chunk, halving the per-engine
  epilogue chain (ACT is otherwise idle) and shortening the kernel tail.
- Outputs collect in [128, 1024] bf16 tiles (two batch blocks) so stores
  are full-rate 2KB-per-partition DMAs; the last block stores per-c
  halves immediately after each staggered epilogue.
"""

import os
import sys
import types

for _p in ("/opt/trn_rl_repo",):
    if _p not in sys.path and os.path.isdir(_p):
        sys.path.append(_p)

import numpy as np
import ml_dtypes

import concourse.bacc as bacc
import concourse.mybir as mybir
from concourse import tile
from concourse.bass_utils import run_bass_kernel_spmd

N_CORES = 8
BATCH = 32768
SHARD = BATCH // N_CORES          # 4096 rows per core
IN = 1024
OUT = 1024
EPS = 1e-6
P = 128                           # SBUF partitions
NB = 512                          # moving free-dim per matmul
NBC = SHARD // NB                 # 8 batch blocks per core
OC = OUT // P                     # 8 output-feature chunks
K8 = 512                          # contraction columns done in fp8
JP = K8 // (2 * P)                # 2 DoubleRow k-pair units (256 each)
KB = (IN - K8) // P               # 4 bf16 k-chunk units (128 each)
NU = JP + KB                      # 6 accumulation units per group

F32 = mybir.dt.float32
BF16 = mybir.dt.bfloat16
FP8 = mybir.dt.float8e4
Alu = mybir.AluOpType
Act = mybir.ActivationFunctionType
DRMODE = mybir.MatmulPerfMode.DoubleRow

# Dummy matmuls bridge the PE from the preamble (~8us) until the first
# real tiles land (~11.3us), keeping the PE busy through the HAM window
# so it reaches 2.4 GHz before (or just as) the real stream starts.  The
# N=64 ones burn fast (53ns cold) to accumulate busy-time; the N=512
# ones stretch coverage.  The real stream then ramps while the rest of
# the head streams in (loads are issued in consumption order, so every
# remaining stall stays well under the 3.4us HAM re-throttle window).
WARM_SMALL = 24
WARM_BIG = 2

# batch-block DMA stages: blocks 0 and 1 load individually so the stream
# can start as soon as ~0.25MB of x has landed; the rest follows in
# per-queue FIFO order behind the head.
X8_STAGES = [(0, 1), (1, 2), (2, 5), (5, 8)]
XB_STAGES = [(0, 1), (1, 2), (2, 5), (5, 8)]


def _install_trace_shim():
    """antenv.axon_hooks is absent in this image; recreate it so
    run_bass_kernel_spmd(trace=True) can capture NTFF profiles."""
    try:
        import antenv.axon_hooks  # noqa: F401
        return
    except ImportError:
        pass
    try:
        import trn_agent_boot.trn_boot as tb
        hooks = types.ModuleType("antenv.axon_hooks")
        hooks._hook = tb._ntff_profile_via_ctypes("/opt/axon/libaxon_pjrt.so")
        hooks.get_axon_ntff_profile_hook = lambda: hooks._hook
        hooks.set_axon_ntff_profile_hook = lambda h: setattr(hooks, "_hook", h)
        sys.modules["antenv.axon_hooks"] = hooks
        import concourse.bass_utils as bass_utils
        bass_utils.upload_artifacts = lambda tmpdir: f"file://{tmpdir}"
    except Exception:
        pass


def build_program():
    nc = bacc.Bacc("TRN2", target_bir_lowering=False, debug=False,
                   num_devices=N_CORES)

    # x8: fp8 part of x^T, row j*128+p holds k=256j+128i+p, laid out per
    # batch block as [nb][i][nn] so DMA slabs are contiguous and matmul
    # rhs slices are 3D [128, 2, 512] DoubleRow APs.
    x8_d = nc.dram_tensor("x8", [JP * P, NBC * 2 * NB], FP8,
                          kind="ExternalInput")
    xb_d = nc.dram_tensor("xb", [KB * P, SHARD], BF16, kind="ExternalInput")
    # st: fp8 sign(W)^T for k<512, row j*128+p, cols [i][o]
    st_d = nc.dram_tensor("st", [JP * P, 2 * OUT], FP8, kind="ExternalInput")
    # wt: fp8 sign(W)^T for k>=512 (+-1 is exact in e4m3; the moving rhs
    # stays bf16 so the matmul still runs at the 1-column/cycle bf16 rate)
    wt_d = nc.dram_tensor("wt", [KB * P, OUT], FP8, kind="ExternalInput")
    sc_d = nc.dram_tensor("sc", [OUT], F32, kind="ExternalInput")
    b_d = nc.dram_tensor("b", [OUT], F32, kind="ExternalInput")
    yt_d = nc.dram_tensor("yt", [OUT, SHARD], BF16, kind="ExternalOutput")

    with tile.TileContext(nc) as tc:
        with (
            tc.tile_pool(name="w_pool", bufs=1) as w_pool,
            tc.tile_pool(name="x_pool", bufs=1) as x_pool,
            tc.tile_pool(name="misc", bufs=1) as misc,
            tc.tile_pool(name="ps", bufs=8, space="PSUM") as ps_pool,
            tc.tile_pool(name="yo_pool", bufs=8) as yo_pool,
        ):
            # ---- PE warm-up (no input deps; covers the HAM window while
            # the head DMAs land so the real stream starts at 2.4 GHz)
            warm = misc.tile([P, NB], BF16, tag="warm", name="warm")
            nc.vector.memset(warm[:], 0.0)
            wps = ps_pool.tile([P, NB], F32, tag="ps", name="wps")
            for _ in range(WARM_SMALL):
                nc.tensor.matmul(wps[:, 0:64], warm[:, 0:P], warm[:, 0:64],
                                 start=True, stop=True)
            for _ in range(WARM_BIG):
                nc.tensor.matmul(wps[:], warm[:, 0:P], warm[:],
                                 start=True, stop=True)

            # ---- tiles
            # st is split into two out-halves per j so the first sweep can
            # start after only 128KB of sign data has landed.
            st8 = [[w_pool.tile([P, 2, OUT // 2], FP8, tag=f"st{j}_{oh}",
                                name=f"st{j}_{oh}") for oh in range(2)]
                   for j in range(JP)]
            wt = [w_pool.tile([P, OUT], FP8, tag=f"wt{m}", name=f"wt{m}")
                  for m in range(KB)]
            x8t = [[x_pool.tile([P, 2 * (b1 - b0), NB], FP8,
                                tag=f"x8_{j}_{si}", name=f"x8_{j}_{si}")
                    for si, (b0, b1) in enumerate(X8_STAGES)]
                   for j in range(JP)]
            xbt = [[x_pool.tile([P, (b1 - b0) * NB], BF16,
                                tag=f"xb{m}_{si}", name=f"xb{m}_{si}")
                    for si, (b0, b1) in enumerate(XB_STAGES)]
                   for m in range(KB)]
            scol = misc.tile([P, OC], F32, tag="scol", name="scol")
            bcol = misc.tile([P, OC], F32, tag="bcol", name="bcol")

            def load_x8(j, si, eng):
                b0, b1 = X8_STAGES[si]
                eng.dma_start(x8t[j][si][:],
                              x8_d.ap()[j * P:(j + 1) * P,
                                        b0 * 2 * NB:b1 * 2 * NB])

            def load_xb(m, si, eng):
                b0, b1 = XB_STAGES[si]
                eng.dma_start(xbt[m][si][:],
                              xb_d.ap()[m * P:(m + 1) * P, b0 * NB:b1 * NB])

            def load_st(j, oh, eng):
                eng.dma_start(st8[j][oh][:],
                              st_d.ap()[j * P:(j + 1) * P,
                                        oh * OUT:(oh + 1) * OUT])

            # ---- head DMAs across the three DMA-capable queues (sync /
            # gpsimd / scalar) in exact consumption order of the skewed
            # block-0 waves; per-queue FIFO keeps later items from
            # stealing bandwidth from earlier ones.
            load_st(0, 0, nc.sync)
            load_st(0, 1, nc.gpsimd)
            load_x8(0, 0, nc.scalar)
            nc.sync.dma_start(wt[0][:], wt_d.ap()[0:P, :])
            load_xb(0, 0, nc.gpsimd)
            load_st(1, 1, nc.scalar)
            nc.sync.dma_start(wt[1][:], wt_d.ap()[P:2 * P, :])
            load_xb(1, 0, nc.gpsimd)
            nc.scalar.dma_start(wt[2][:], wt_d.ap()[2 * P:3 * P, :])
            load_st(1, 0, nc.gpsimd)
            load_x8(1, 0, nc.sync)
            load_xb(2, 0, nc.scalar)
            load_xb(3, 0, nc.gpsimd)
            nc.sync.dma_start(wt[3][:], wt_d.ap()[3 * P:4 * P, :])
            nc.gpsimd.dma_start(scol[:], sc_d.ap().rearrange("(c p) -> p c", p=P))
            nc.gpsimd.dma_start(bcol[:], b_d.ap().rearrange("(c p) -> p c", p=P))

            # block 1, then the later stages; scalar takes the mid x8
            # stages (its queue clears early for ACT epilogues + stores).
            load_x8(0, 1, nc.sync)
            load_x8(1, 1, nc.gpsimd)
            for m in range(KB):
                load_xb(m, 1, (nc.sync, nc.gpsimd)[m % 2])
            load_x8(0, 2, nc.scalar)
            load_x8(1, 2, nc.scalar)
            for m in range(KB):
                load_xb(m, 2, (nc.sync, nc.gpsimd)[m % 2])
            load_x8(0, 3, nc.sync)
            load_x8(1, 3, nc.gpsimd)
            for m in range(KB):
                load_xb(m, 3, (nc.sync, nc.gpsimd)[m % 2])

            def stage_of(stages, n):
                for si, (b0, b1) in enumerate(stages):
                    if b0 <= n < b1:
                        return si, n - b0
                raise AssertionError(n)

            def rhs_for(u, n):
                if u < JP:
                    si, ln = stage_of(X8_STAGES, n)
                    return x8t[u][si][:, 2 * ln:2 * ln + 2, :]
                si, ln = stage_of(XB_STAGES, n)
                return xbt[u - JP][si][:, ln * NB:(ln + 1) * NB]

            yo_cur = [None] * OC

            def epilogue(n, c, ps):
                half = n % 2
                if half == 0:
                    yo_cur[c] = yo_pool.tile([P, 2 * NB], BF16, tag="yo",
                                             name=f"yo{n}_{c}")
                yo = yo_cur[c]
                dst = yo[:, half * NB:(half + 1) * NB]
                if c % 2 == 0:
                    nc.vector.tensor_scalar(dst, ps[:], scol[:, c:c + 1],
                                            bcol[:, c:c + 1], Alu.mult, Alu.add)
                else:
                    nc.scalar.activation(dst, ps[:], Act.Identity,
                                         bias=bcol[:, c:c + 1],
                                         scale=scol[:, c:c + 1])
                if n == NBC - 2:
                    # penultimate block: store its half immediately so it
                    # overlaps the last block's compute
                    nc.scalar.dma_start(
                        yt_d.ap()[c * P:(c + 1) * P, n * NB:(n + 1) * NB],
                        yo[:, 0:NB])
                elif n == NBC - 1:
                    # last block: per-c half stores fire as each staggered
                    # epilogue completes -> short kernel tail
                    eng = nc.sync if c % 2 == 1 else nc.scalar
                    eng.dma_start(
                        yt_d.ap()[c * P:(c + 1) * P, n * NB:(n + 1) * NB],
                        yo[:, NB:2 * NB])
                elif half == 1:
                    eng = nc.scalar if c % 2 == 1 else nc.sync
                    eng.dma_start(
                        yt_d.ap()[c * P:(c + 1) * P,
                                  (n - 1) * NB:(n + 1) * NB],
                        yo[:])

            # Per-bank unit order: DoubleRow MMs at slots 0 and 3 so two DR
            # MMs are never issued back-to-back (a DR pair costs an extra
            # ~30ns drain gap when adjacent).
            UORDER = (0, 2, 3, 1, 4, 5)

            def mm(s, c, n, ps):
                u = UORDER[s]
                if u < JP:
                    lhsT = st8[u][c // 4][:, :, (c % 4) * P:(c % 4 + 1) * P]
                    nc.tensor.matmul(ps[:], lhsT,
                                     rhs_for(u, n), start=(s == 0), stop=False,
                                     perf_mode=DRMODE)
                else:
                    nc.tensor.matmul(ps[:], wt[u - JP][:, c * P:(c + 1) * P],
                                     rhs_for(u, n), start=(s == 0),
                                     stop=(s == NU - 1))

            # ---- main loop: skewed waves.  MM(unit u, out-chunk c) goes
            # at wave u+c; each bank's 6-MM accumulation finishes one wave
            # after the previous bank's, so epilogues stagger and PSUM
            # banks are long free before block n+1 reuses them.
            for n in range(NBC):
                yps = [ps_pool.tile([P, NB], F32, tag="ps", name=f"yp{n}_{c}")
                       for c in range(OC)]
                for wv in range(NU + OC - 1):
                    for c in range(OC):
                        u = wv - c
                        if 0 <= u < NU:
                            mm(u, c, n, yps[c])
                            if u == NU - 1:
                                epilogue(n, c, yps[c])

    nc.compile()
    return nc


_NC = None


def _get_program():
    global _NC
    if _NC is None:
        _NC = build_program()
    return _NC


def kernel(x: np.ndarray, W: np.ndarray, b: np.ndarray) -> np.ndarray:
    assert x.shape == (BATCH, IN) and W.shape == (OUT, IN) and b.shape == (OUT,)
    nc = _get_program()

    Wf = np.asarray(W, dtype=np.float32)
    sgnT = np.where(Wf >= 0, np.float32(1.0), np.float32(-1.0)).T  # [in, out]
    # st cols are [oh (out half)][i (k subtile)][o'], so each out-half is
    # one contiguous 1KB-per-partition DMA
    st_pack = np.ascontiguousarray(
        sgnT[:K8].reshape(JP, 2, P, 2, OUT // 2).transpose(0, 2, 3, 1, 4)
        .reshape(JP * P, 2 * OUT)).astype(ml_dtypes.float8_e4m3)
    wt_pack = np.ascontiguousarray(sgnT[K8:]).astype(ml_dtypes.float8_e4m3)
    sc = np.maximum(np.abs(Wf).mean(axis=1), EPS).astype(np.float32)
    b32 = np.ascontiguousarray(np.asarray(b, dtype=np.float32))

    in_maps = []
    for c in range(N_CORES):
        xt = x[c * SHARD:(c + 1) * SHARD].T      # [in, n] view
        x8 = xt[:K8].astype(ml_dtypes.float8_e4m3)
        x8 = np.ascontiguousarray(
            x8.reshape(JP, 2, P, NBC, NB).transpose(0, 2, 3, 1, 4)
            .reshape(JP * P, NBC * 2 * NB))
        xb = np.ascontiguousarray(xt[K8:]).astype(ml_dtypes.bfloat16)
        in_maps.append({"x8": x8, "xb": xb, "st": st_pack, "wt": wt_pack,
                        "sc": sc, "b": b32})

    trace = bool(int(os.environ.get("BINLIN_TRACE", "0")))
    if trace:
        _install_trace_shim()
    res = run_bass_kernel_spmd(nc, in_maps, core_ids=list(range(N_CORES)),
                               trace=trace)
    if trace and res.exec_time_ns is not None:
        print(f"HW exec time: {res.exec_time_ns} ns", flush=True)

    y = np.empty((BATCH, OUT), dtype=np.float32)
    for c in range(N_CORES):
        y[c * SHARD:(c + 1) * SHARD] = res.results[c]["yt"].T.astype(np.float32)
    return y



# revision 4
# speedup vs baseline: 1.0022x; 1.0022x over previous
"""BinaryLinear kernel for Trainium2, data-parallel over 8 NeuronCores.

Computes y = x @ (sign(W) * scale).T + b where
  sign(w) = +1 if w >= 0 else -1
  scale_o = max(mean_i |W[o,i]|, 1e-6)           (per output row)

Strategy
--------
- Shard batch (32768) across 8 cores -> 4096 rows/core; replicate weights.
- sign(W) and scale are computed on the HOST (scale from full-precision W,
  so that error source is gone entirely); the device only does matmuls and
  a fused scale*psum+bias epilogue.
- Mixed precision split of the 1024-long contraction, chosen so the
  measured max-rel error (1.79e-2) stays under the 2e-2 gate:
    k in [0,512):   x in fp8 e4m3, sign in fp8, matmul in DoubleRow perf
                    mode (two fp8 weights per PE cell -> K=256 per MM,
                    2x contraction rate: the PE consumes the rhs pair at
                    2 fp8/partition/cycle)
    k in [512,1024): x in bf16, sign in fp8 (exact +-1; mixed-dtype
                    lhsT fp8 x rhs bf16 runs at the full bf16 rate and
                    halves the sign-matrix DMA), regular matmuls
  Per (batch-block, out-block) PSUM group: 2 DoubleRow MMs + 4 bf16 MMs
  accumulate f32 into one PSUM bank; trace-measured steady state is a
  663 ns period per {DR, bf16, bf16} triple = ~96.5% PE-array occupancy,
  i.e. the stream is at the accuracy-constrained PE floor (int8, which
  would beat e4m3's error at the same DR rate, is rejected by the BIR
  verifier; e3m4 has no DoubleRow).
- Skewed wave schedule: MM(unit u, out-block c) issues at wave u+c, so
  the 8 PSUM banks finish staggered ~1 wave apart instead of all in the
  final sweep.  Epilogues then never queue up and bank recycling never
  stalls the PE.
- Head DMAs fan out over FOUR engine queues (sync/scalar/gpsimd/vector)
  in strict first-need order of the skewed block-0 waves.  Per-queue
  head bandwidth is only ~80 GB/s (descriptor-paced), so each queue
  carries one first-wave tile, then second-wave tiles, etc.  Later
  stages are j-/m-combined into wide slabs (2-6 KB per partition) so
  the bulk of x streams at full DMA rate behind the head.
- PE warm-up matmuls with no input deps bridge the preamble (~6 us) to
  the first real MMs (~10 us) with no PE-idle gap, so the HAM clock gate
  reaches 8/8 during the warmups and never re-throttles mid-stream
  (the v1 kernel lost ~3.4 us to a 6.8 us K=4/8 window triggered by a
  2.4 us idle gap at the warmup->stream handoff).
- Epilogues alternate DVE / ACT per out-chunk, halving the per-engine
  epilogue chain; outputs collect in [128, 1024] bf16 tiles (two batch
  blocks) for full-rate 2KB-per-partition stores.  The last block's
  per-c stores fan out over all four queues to shorten the kernel tail.
"""

import os
import sys
import types

for _p in ("/opt/trn_rl_repo",):
    if _p not in sys.path and os.path.isdir(_p):
        sys.path.append(_p)

import numpy as np
import ml_dtypes

import concourse.bacc as bacc
import concourse.mybir as mybir
from concourse import tile
from concourse.bass_utils import run_bass_kernel_spmd

N_CORES = 8
BATCH = 32768
SHARD = BATCH // N_CORES          # 4096 rows per core
IN = 1024
OUT = 1024
EPS = 1e-6
P = 128                           # SBUF partitions
NB = 512                          # moving free-dim per matmul
NBC = SHARD // NB                 # 8 batch blocks per core
OC = OUT // P                     # 8 output-feature chunks
K8 = 512                          # contraction columns done in fp8
JP = K8 // (2 * P)                # 2 DoubleRow k-pair units (256 each)
KB = (IN - K8) // P               # 4 bf16 k-chunk units (128 each)
NU = JP + KB                      # 6 accumulation units per group

F32 = mybir.dt.float32
BF16 = mybir.dt.bfloat16
FP8 = mybir.dt.float8e4
Alu = mybir.AluOpType
Act = mybir.ActivationFunctionType
DRMODE = mybir.MatmulPerfMode.DoubleRow

# Dummy matmuls bridge the PE from the preamble (~6us) until the first
# real tiles land (~10us), keeping the PE busy through the HAM window so
# it reaches 2.4 GHz before the real stream starts and never re-gates.
WARM_SMALL = 24
WARM_BIG = 2

# batch-block DMA stages: block 0 loads as per-unit 128KB tiles (one per
# queue, strict need order) so the stream starts as soon as possible;
# later stages are j-/m-combined slabs that ride behind the head.
X8_STAGES = [(0, 1), (1, 2), (2, 5), (5, 8)]
XB_STAGES = [(0, 1), (1, 2), (2, 5), (5, 8)]


def _install_trace_shim():
    """antenv.axon_hooks is absent in this image; recreate it so
    run_bass_kernel_spmd(trace=True) can capture NTFF profiles."""
    try:
        import antenv.axon_hooks  # noqa: F401
        return
    except ImportError:
        pass
    try:
        import trn_agent_boot.trn_boot as tb
        hooks = types.ModuleType("antenv.axon_hooks")
        hooks._hook = tb._ntff_profile_via_ctypes("/opt/axon/libaxon_pjrt.so")
        hooks.get_axon_ntff_profile_hook = lambda: hooks._hook
        hooks.set_axon_ntff_profile_hook = lambda h: setattr(hooks, "_hook", h)
        sys.modules["antenv.axon_hooks"] = hooks
        import concourse.bass_utils as bass_utils
        bass_utils.upload_artifacts = lambda tmpdir: f"file://{tmpdir}"
    except Exception:
        pass


def build_program():
    nc = bacc.Bacc("TRN2", target_bir_lowering=False, debug=False,
                   num_devices=N_CORES)

    # x8: fp8 part of x^T, row j*128+p holds k=256j+128i+p, laid out per
    # batch block as [nb][i][nn] so DMA slabs are contiguous and matmul
    # rhs slices are 3D [128, 2, 512] DoubleRow APs.
    x8_d = nc.dram_tensor("x8", [JP * P, NBC * 2 * NB], FP8,
                          kind="ExternalInput")
    xb_d = nc.dram_tensor("xb", [KB * P, SHARD], BF16, kind="ExternalInput")
    # st: fp8 sign(W)^T for k<512, row j*128+p, cols [oh][i][o']
    st_d = nc.dram_tensor("st", [JP * P, 2 * OUT], FP8, kind="ExternalInput")
    # wt: fp8 sign(W)^T for k>=512 (+-1 is exact in e4m3; the moving rhs
    # stays bf16 so the matmul still runs at the 1-column/cycle bf16 rate)
    wt_d = nc.dram_tensor("wt", [KB * P, OUT], FP8, kind="ExternalInput")
    sc_d = nc.dram_tensor("sc", [OUT], F32, kind="ExternalInput")
    b_d = nc.dram_tensor("b", [OUT], F32, kind="ExternalInput")
    yt_d = nc.dram_tensor("yt", [OUT, SHARD], BF16, kind="ExternalOutput")

    with tile.TileContext(nc) as tc:
        with (
            tc.tile_pool(name="w_pool", bufs=1) as w_pool,
            tc.tile_pool(name="x_pool", bufs=1) as x_pool,
            tc.tile_pool(name="misc", bufs=1) as misc,
            tc.tile_pool(name="ps", bufs=8, space="PSUM") as ps_pool,
            tc.tile_pool(name="yo_pool", bufs=8) as yo_pool,
        ):
            # ---- PE warm-up (no input deps; covers the HAM window while
            # the head DMAs land so the real stream starts at 2.4 GHz)
            warm = misc.tile([P, NB], BF16, tag="warm", name="warm")
            nc.vector.memset(warm[:], 0.0)
            wps = ps_pool.tile([P, NB], F32, tag="ps", name="wps")
            for _ in range(WARM_SMALL):
                nc.tensor.matmul(wps[:, 0:64], warm[:, 0:P], warm[:, 0:64],
                                 start=True, stop=True)
            for _ in range(WARM_BIG):
                nc.tensor.matmul(wps[:], warm[:, 0:P], warm[:],
                                 start=True, stop=True)

            # ---- tiles
            # st is split into two out-halves per j so the first sweep can
            # start after only 128KB of sign data has landed.
            st8 = [[w_pool.tile([P, 2, OUT // 2], FP8, tag=f"st{j}_{oh}",
                                name=f"st{j}_{oh}") for oh in range(2)]
                   for j in range(JP)]
            wt = [w_pool.tile([P, OUT], FP8, tag=f"wt{m}", name=f"wt{m}")
                  for m in range(KB)]
            # x8 stage 0: per-j 128KB tiles; stages >=1: j-combined slabs
            # [P, 2(j), (b1-b0)*1024] (2-6KB per partition rows).
            x8t0 = [x_pool.tile([P, 2, NB], FP8, tag=f"x80_{j}",
                                name=f"x80_{j}") for j in range(JP)]
            x8c = [None] + [
                x_pool.tile([P, JP, (b1 - b0) * 2 * NB], FP8,
                            tag=f"x8c_{si}", name=f"x8c_{si}")
                for si, (b0, b1) in enumerate(X8_STAGES) if si >= 1]
            # xb stage 0: per-m 128KB tiles; stages >=1: m-paired slabs
            # [P, 2(m), (b1-b0)*NB].
            xbt0 = [x_pool.tile([P, NB], BF16, tag=f"xb0_{m}",
                                name=f"xb0_{m}") for m in range(KB)]
            xbc = [[None] + [
                x_pool.tile([P, 2, (b1 - b0) * NB], BF16,
                            tag=f"xbc{mp}_{si}", name=f"xbc{mp}_{si}")
                for si, (b0, b1) in enumerate(XB_STAGES) if si >= 1]
                for mp in range(KB // 2)]
            scol = misc.tile([P, OC], F32, tag="scol", name="scol")
            bcol = misc.tile([P, OC], F32, tag="bcol", name="bcol")

            def load_x8c(si, eng):
                b0, b1 = X8_STAGES[si]
                src = x8_d.ap()[:, b0 * 2 * NB:b1 * 2 * NB].rearrange(
                    "(j p) c -> p j c", j=JP)
                eng.dma_start(x8c[si][:], src)

            def load_xbc(mp, si, eng):
                b0, b1 = XB_STAGES[si]
                src = xb_d.ap()[mp * 2 * P:(mp + 1) * 2 * P,
                                b0 * NB:b1 * NB].rearrange(
                    "(m p) n -> p m n", m=2)
                eng.dma_start(xbc[mp][si][:], src)

            def load_st(j, oh, eng):
                eng.dma_start(st8[j][oh][:],
                              st_d.ap()[j * P:(j + 1) * P,
                                        oh * OUT:(oh + 1) * OUT])

            # ---- head DMAs: the three DMA-capable queues (sync / scalar
            # / gpsimd), strict first-need order of the skewed block-0
            # waves (UORDER = u0,u2,u3,u1,u4,u5):
            #   w0: st(0,0) x8(j0)   w1: wt0 xb(m0)   w2: wt1 xb(m1)
            #   w3: st(1,0) x8(j1)   w4: wt2 xb(m2) st(0,1)   w5: wt3 xb(m3)
            # slot 1 on each queue
            load_st(0, 0, nc.sync)
            nc.scalar.dma_start(x8t0[0][:],
                                x8_d.ap()[0:P, 0:2 * NB])
            nc.gpsimd.dma_start(wt[0][:], wt_d.ap()[0:P, :])
            # slot 2
            nc.sync.dma_start(xbt0[0][:], xb_d.ap()[0:P, 0:NB])
            nc.scalar.dma_start(wt[1][:], wt_d.ap()[P:2 * P, :])
            nc.gpsimd.dma_start(xbt0[1][:], xb_d.ap()[P:2 * P, 0:NB])
            # slot 3
            load_st(1, 0, nc.sync)
            nc.scalar.dma_start(x8t0[1][:],
                                x8_d.ap()[P:2 * P, 0:2 * NB])
            nc.gpsimd.dma_start(wt[2][:], wt_d.ap()[2 * P:3 * P, :])
            # slot 4
            nc.sync.dma_start(xbt0[2][:], xb_d.ap()[2 * P:3 * P, 0:NB])
            load_st(0, 1, nc.scalar)
            nc.gpsimd.dma_start(wt[3][:], wt_d.ap()[3 * P:4 * P, :])
            # slot 5
            nc.sync.dma_start(xbt0[3][:], xb_d.ap()[3 * P:4 * P, 0:NB])
            load_st(1, 1, nc.scalar)
            load_x8c(1, nc.gpsimd)
            # epilogue scale/bias (tiny; needed from the first epilogue on)
            nc.gpsimd.dma_start(scol[:],
                                sc_d.ap().rearrange("(c p) -> p c", p=P))
            nc.gpsimd.dma_start(bcol[:],
                                b_d.ap().rearrange("(c p) -> p c", p=P))
            # stage 1 (block 1) slabs, then the big stage-2/3 slabs
            load_xbc(0, 1, nc.sync)
            load_xbc(1, 1, nc.scalar)
            load_x8c(2, nc.gpsimd)
            load_xbc(0, 2, nc.sync)
            load_xbc(1, 2, nc.scalar)
            load_x8c(3, nc.gpsimd)
            load_xbc(0, 3, nc.sync)
            load_xbc(1, 3, nc.scalar)

            def stage_of(stages, n):
                for si, (b0, b1) in enumerate(stages):
                    if b0 <= n < b1:
                        return si, n - b0
                raise AssertionError(n)

            def rhs_for(u, n):
                if u < JP:
                    si, ln = stage_of(X8_STAGES, n)
                    if si == 0:
                        return x8t0[u][:]
                    return x8c[si][:, u, ln * 2 * NB:(ln + 1) * 2 * NB] \
                        .rearrange("p (i n) -> p i n", i=2)
                m = u - JP
                si, ln = stage_of(XB_STAGES, n)
                if si == 0:
                    return xbt0[m][:]
                return xbc[m // 2][si][:, m % 2, ln * NB:(ln + 1) * NB]

            yo_cur = [None] * OC
            TAIL_ENGS = None  # set below

            def epilogue(n, c, ps):
                half = n % 2
                if half == 0:
                    yo_cur[c] = yo_pool.tile([P, 2 * NB], BF16, tag="yo",
                                             name=f"yo{n}_{c}")
                yo = yo_cur[c]
                dst = yo[:, half * NB:(half + 1) * NB]
                if c % 2 == 0:
                    nc.vector.tensor_scalar(dst, ps[:], scol[:, c:c + 1],
                                            bcol[:, c:c + 1], Alu.mult, Alu.add)
                else:
                    nc.scalar.activation(dst, ps[:], Act.Identity,
                                         bias=bcol[:, c:c + 1],
                                         scale=scol[:, c:c + 1])
                if n == NBC - 2:
                    # penultimate block: store its half immediately so it
                    # overlaps the last block's compute
                    nc.scalar.dma_start(
                        yt_d.ap()[c * P:(c + 1) * P, n * NB:(n + 1) * NB],
                        yo[:, 0:NB])
                elif n == NBC - 1:
                    # last block: per-c half stores fan out over the three
                    # DMA queues as each staggered epilogue completes ->
                    # short kernel tail
                    eng = (nc.sync, nc.scalar, nc.gpsimd)[c % 3]
                    eng.dma_start(
                        yt_d.ap()[c * P:(c + 1) * P, n * NB:(n + 1) * NB],
                        yo[:, NB:2 * NB])
                elif half == 1:
                    eng = nc.scalar if c % 2 == 1 else nc.sync
                    eng.dma_start(
                        yt_d.ap()[c * P:(c + 1) * P,
                                  (n - 1) * NB:(n + 1) * NB],
                        yo[:])

            # Per-bank unit order: DoubleRow MMs at slots 0 and 3 so two DR
            # MMs are never issued back-to-back (a DR pair costs an extra
            # ~30ns drain gap when adjacent).
            UORDER = (0, 2, 3, 1, 4, 5)

            def mm(s, c, n, ps):
                u = UORDER[s]
                if u < JP:
                    lhsT = st8[u][c // 4][:, :, (c % 4) * P:(c % 4 + 1) * P]
                    nc.tensor.matmul(ps[:], lhsT,
                                     rhs_for(u, n), start=(s == 0), stop=False,
                                     perf_mode=DRMODE)
                else:
                    nc.tensor.matmul(ps[:], wt[u - JP][:, c * P:(c + 1) * P],
                                     rhs_for(u, n), start=(s == 0),
                                     stop=(s == NU - 1))

            # ---- main loop: skewed waves.  MM(unit u, out-chunk c) goes
            # at wave u+c; each bank's 6-MM accumulation finishes one wave
            # after the previous bank's, so epilogues stagger and PSUM
            # banks are long free before block n+1 reuses them.
            for n in range(NBC):
                yps = [ps_pool.tile([P, NB], F32, tag="ps", name=f"yp{n}_{c}")
                       for c in range(OC)]
                for wv in range(NU + OC - 1):
                    for c in range(OC):
                        u = wv - c
                        if 0 <= u < NU:
                            mm(u, c, n, yps[c])
                            if u == NU - 1:
                                epilogue(n, c, yps[c])

    nc.compile()
    return nc


_NC = None


def _get_program():
    global _NC
    if _NC is None:
        _NC = build_program()
    return _NC


def kernel(x: np.ndarray, W: np.ndarray, b: np.ndarray) -> np.ndarray:
    assert x.shape == (BATCH, IN) and W.shape == (OUT, IN) and b.shape == (OUT,)
    nc = _get_program()

    Wf = np.asarray(W, dtype=np.float32)
    sgnT = np.where(Wf >= 0, np.float32(1.0), np.float32(-1.0)).T  # [in, out]
    # st cols are [oh (out half)][i (k subtile)][o'], so each out-half is
    # one contiguous 1KB-per-partition DMA
    st_pack = np.ascontiguousarray(
        sgnT[:K8].reshape(JP, 2, P, 2, OUT // 2).transpose(0, 2, 3, 1, 4)
        .reshape(JP * P, 2 * OUT)).astype(ml_dtypes.float8_e4m3)
    wt_pack = np.ascontiguousarray(sgnT[K8:]).astype(ml_dtypes.float8_e4m3)
    sc = np.maximum(np.abs(Wf).mean(axis=1), EPS).astype(np.float32)
    b32 = np.ascontiguousarray(np.asarray(b, dtype=np.float32))

    in_maps = []
    for c in range(N_CORES):
        xt = x[c * SHARD:(c + 1) * SHARD].T      # [in, n] view
        x8 = xt[:K8].astype(ml_dtypes.float8_e4m3)
        x8 = np.ascontiguousarray(
            x8.reshape(JP, 2, P, NBC, NB).transpose(0, 2, 3, 1, 4)
            .reshape(JP * P, NBC * 2 * NB))
        xb = np.ascontiguousarray(xt[K8:]).astype(ml_dtypes.bfloat16)
        in_maps.append({"x8": x8, "xb": xb, "st": st_pack, "wt": wt_pack,
                        "sc": sc, "b": b32})

    trace = bool(int(os.environ.get("BINLIN_TRACE", "0")))
    if trace:
        _install_trace_shim()
    res = run_bass_kernel_spmd(nc, in_maps, core_ids=list(range(N_CORES)),
                               trace=trace)
    if trace and res.exec_time_ns is not None:
        print(f"HW exec time: {res.exec_time_ns} ns", flush=True)

    y = np.empty((BATCH, OUT), dtype=np.float32)
    for c in range(N_CORES):
        y[c * SHARD:(c + 1) * SHARD] = res.results[c]["yt"].T.astype(np.float32)
    return y


# revision 5
# speedup vs baseline: 1.0143x; 1.0120x over previous
"""BinaryLinear kernel for Trainium2, data-parallel over 8 NeuronCores.

Computes y = x @ (sign(W) * scale).T + b where
  sign(w) = +1 if w >= 0 else -1
  scale_o = max(mean_i |W[o,i]|, 1e-6)           (per output row)

Strategy
--------
- Shard batch (32768) across 8 cores -> 4096 rows/core; replicate weights.
- sign(W) and scale are computed on the HOST (scale from full-precision W,
  so that error source is gone entirely); the device only does matmuls and
  a fused scale*psum+bias epilogue.
- Mixed precision split of the 1024-long contraction, chosen so the
  measured max-rel error (1.79e-2) stays under the 2e-2 gate:
    k in [0,512):   x in fp8 e4m3, sign in fp8, matmul in DoubleRow perf
                    mode (2 fp8 weights per PE cell, rhs pair consumed at
                    2 fp8/partition/cycle -> K=256 per 512-cycle MM)
    k in [512,1024): x in bf16, sign in fp8 (exact +-1; mixed-dtype
                    lhsT fp8 x rhs bf16 runs at the full bf16 rate)
  Per (batch-block, out-block) PSUM group: 2 DR MMs + 4 bf16 MMs; the
  trace-measured steady state is a 663 ns period per {DR, bf16, bf16}
  triple = ~96.5% PE-array occupancy, i.e. the stream is at the
  accuracy-constrained PE floor (int8, which would beat e4m3 error at
  the same DR rate, is rejected by the BIR verifier; e3m4 has no
  DoubleRow; more fp8 columns breaks the 2e-2 gate at 2.5e-2).
- Block 0 runs its two DoubleRow c-sweeps FIRST (phase A: 16 DR MMs
  needing only sign+x8 fp8 tiles, 768KB of DMA), then the bf16 units in
  skewed waves (phase B).  The PE engine queue is strict FIFO, so
  without this the first bf16 MM - whose wt/xb tiles land ~3us after
  the fp8 head - stalls the whole stream behind it.  Phase A is a
  ~3.8us runway that covers the bf16 head DMA and keeps the PE busy
  through the HAM clock-gate window (continuous activity from the
  warmups -> K=8/8 early, no mid-stream re-throttle).
- Blocks 1..7 use the skewed wave schedule: MM(unit u, out-chunk c) at
  wave u+c, so the 8 PSUM banks finish staggered ~1 wave apart,
  epilogues never queue up, and bank recycling never stalls the PE.
- Head DMAs ride the three DMA-capable queues (sync/scalar/gpsimd) in
  strict first-need order; per-queue head throughput is only ~90 GB/s,
  so ordering is everything.  x data is packed block-major on the host
  ([p, block, ...unit...]) so every DMA stage is one contiguous 2-6KB
  -per-partition slab; scale/bias are host-packed into a single [128,16]
  tile (the per-element gather it replaces cost 2.3us of queue time).
- Epilogues alternate DVE / ACT per out-chunk; outputs collect in
  [128, 1024] bf16 tiles (two batch blocks) for full-rate 2KB-per-
  partition stores; the last block's per-c stores fan out over the
  three queues to shorten the kernel tail.
"""

import os
import sys
import types

for _p in ("/opt/trn_rl_repo",):
    if _p not in sys.path and os.path.isdir(_p):
        sys.path.append(_p)

import numpy as np
import ml_dtypes

import concourse.bacc as bacc
import concourse.mybir as mybir
from concourse import tile
from concourse.bass_utils import run_bass_kernel_spmd

N_CORES = 8
BATCH = 32768
SHARD = BATCH // N_CORES          # 4096 rows per core
IN = 1024
OUT = 1024
EPS = 1e-6
P = 128                           # SBUF partitions
NB = 512                          # moving free-dim per matmul
NBC = SHARD // NB                 # 8 batch blocks per core
OC = OUT // P                     # 8 output-feature chunks
K8 = 512                          # contraction columns done in fp8
JP = K8 // (2 * P)                # 2 DoubleRow k-pair units (256 each)
KB = (IN - K8) // P               # 4 bf16 k-chunk units (128 each)
NU = JP + KB                      # 6 accumulation units per group

F32 = mybir.dt.float32
BF16 = mybir.dt.bfloat16
FP8 = mybir.dt.float8e4
Alu = mybir.AluOpType
Act = mybir.ActivationFunctionType
DRMODE = mybir.MatmulPerfMode.DoubleRow

# Dummy matmuls bridge the PE from the preamble (~7us) until the first
# real tiles land (~10us): continuous PE activity into phase A keeps the
# HAM clock-gate busy-window filled so the PE un-throttles early.
WARM_SMALL = 24
WARM_BIG = 2

# batch-block DMA stages (one contiguous slab per stage per tensor)
X8_STAGES = [(0, 1), (1, 2), (2, 5), (5, 8)]
XB_STAGES = [(0, 1), (1, 2), (2, 5), (5, 8)]


def _install_trace_shim():
    """antenv.axon_hooks is absent in this image; recreate it so
    run_bass_kernel_spmd(trace=True) can capture NTFF profiles."""
    try:
        import antenv.axon_hooks  # noqa: F401
        return
    except ImportError:
        pass
    try:
        import trn_agent_boot.trn_boot as tb
        hooks = types.ModuleType("antenv.axon_hooks")
        hooks._hook = tb._ntff_profile_via_ctypes("/opt/axon/libaxon_pjrt.so")
        hooks.get_axon_ntff_profile_hook = lambda: hooks._hook
        hooks.set_axon_ntff_profile_hook = lambda h: setattr(hooks, "_hook", h)
        sys.modules["antenv.axon_hooks"] = hooks
        import concourse.bass_utils as bass_utils
        bass_utils.upload_artifacts = lambda tmpdir: f"file://{tmpdir}"
    except Exception:
        pass


def build_program():
    nc = bacc.Bacc("TRN2", target_bir_lowering=False, debug=False,
                   num_devices=N_CORES)

    # x8: fp8 part of x^T packed block-major: row p, block nb at byte
    # nb*2048, layout [j][i][nn] inside -> every stage is one contiguous
    # slab and rhs slices are [128, 2, 512] DoubleRow APs.
    x8_d = nc.dram_tensor("x8", [P, NBC * 2048], FP8, kind="ExternalInput")
    # xb: bf16 part of x^T packed [p][mp][nb][mm][nn] -> per-(mp, stage)
    # contiguous slabs with 2KB+ rows.
    xb_d = nc.dram_tensor("xb", [P, 2 * NBC * 1024], BF16,
                          kind="ExternalInput")
    # st: fp8 sign(W)^T for k<512, row j*128+p, cols [oh][i][o']
    st_d = nc.dram_tensor("st", [JP * P, 2 * OUT], FP8, kind="ExternalInput")
    # wt: fp8 sign(W)^T for k>=512 (+-1 exact in e4m3; moving rhs stays
    # bf16 so the matmul runs at the 1-column/cycle bf16 rate)
    wt_d = nc.dram_tensor("wt", [KB * P, OUT], FP8, kind="ExternalInput")
    # scb: host-packed scale/bias columns [p, c] / [p, OC+c]
    scb_d = nc.dram_tensor("scb", [P, 2 * OC], F32, kind="ExternalInput")
    yt_d = nc.dram_tensor("yt", [OUT, SHARD], BF16, kind="ExternalOutput")

    with tile.TileContext(nc) as tc:
        with (
            tc.tile_pool(name="w_pool", bufs=1) as w_pool,
            tc.tile_pool(name="x_pool", bufs=1) as x_pool,
            tc.tile_pool(name="misc", bufs=1) as misc,
            tc.tile_pool(name="ps", bufs=8, space="PSUM") as ps_pool,
            tc.tile_pool(name="yo_pool", bufs=8) as yo_pool,
        ):
            # ---- PE warm-up (no input deps)
            warm = misc.tile([P, NB], BF16, tag="warm", name="warm")
            nc.vector.memset(warm[:], 0.0)
            wps = ps_pool.tile([P, NB], F32, tag="ps", name="wps")
            for _ in range(WARM_SMALL):
                nc.tensor.matmul(wps[:, 0:64], warm[:, 0:P], warm[:, 0:64],
                                 start=True, stop=True)
            for _ in range(WARM_BIG):
                nc.tensor.matmul(wps[:], warm[:, 0:P], warm[:],
                                 start=True, stop=True)

            # ---- tiles
            st8 = [[w_pool.tile([P, 2, OUT // 2], FP8, tag=f"st{j}_{oh}",
                                name=f"st{j}_{oh}") for oh in range(2)]
                   for j in range(JP)]
            wt = [w_pool.tile([P, OUT], FP8, tag=f"wt{m}", name=f"wt{m}")
                  for m in range(KB)]
            # x8 stage 0 split per j (phase A consumes j0 first); stages
            # >=1 are single [P, (b1-b0)*2048] slabs.
            x8s0 = [x_pool.tile([P, 2 * NB], FP8, tag=f"x80_{j}",
                                name=f"x80_{j}") for j in range(JP)]
            x8s = [None] + [
                x_pool.tile([P, (b1 - b0) * 2048], FP8,
                            tag=f"x8s_{si}", name=f"x8s_{si}")
                for si, (b0, b1) in enumerate(X8_STAGES) if si >= 1]
            # xb per (mp=unit-pair, stage) slabs [P, (b1-b0)*1024] bf16
            xbp = [[x_pool.tile([P, (b1 - b0) * 1024], BF16,
                                tag=f"xb{mp}_{si}", name=f"xb{mp}_{si}")
                    for si, (b0, b1) in enumerate(XB_STAGES)]
                   for mp in range(KB // 2)]
            scb = misc.tile([P, 2 * OC], F32, tag="scb", name="scb")

            def load_x8s(si, eng):
                b0, b1 = X8_STAGES[si]
                if si == 0:
                    for j in range(JP):
                        eng.dma_start(x8s0[j][:],
                                      x8_d.ap()[:, j * 1024:(j + 1) * 1024])
                else:
                    eng.dma_start(x8s[si][:],
                                  x8_d.ap()[:, b0 * 2048:b1 * 2048])

            def load_xbp(mp, si, eng):
                b0, b1 = XB_STAGES[si]
                base = mp * NBC * 1024
                eng.dma_start(xbp[mp][si][:],
                              xb_d.ap()[:, base + b0 * 1024:base + b1 * 1024])

            def load_st(j, oh, eng):
                eng.dma_start(st8[j][oh][:],
                              st_d.ap()[j * P:(j + 1) * P,
                                        oh * OUT:(oh + 1) * OUT])

            # ---- head DMAs, three queues, strict first-need order.
            # Phase A (from ~10us): st(0,0)+x8j0, st(0,1)@w4, st(1,0)+
            # x8j1@w8, st(1,1)@w12.  Phase B (from ~14us): wt0+xbp0@w0,
            # wt1@w2, wt2+xbp1@w4(m2 at w2 of deeper c), wt3@w6, scb for
            # the first epilogue (~15us).
            load_st(0, 0, nc.sync)
            nc.scalar.dma_start(x8s0[0][:], x8_d.ap()[:, 0:1024])
            nc.gpsimd.dma_start(wt[0][:], wt_d.ap()[0:P, :])
            load_st(0, 1, nc.sync)
            nc.scalar.dma_start(x8s0[1][:], x8_d.ap()[:, 1024:2048])
            load_st(1, 0, nc.gpsimd)
            load_xbp(0, 0, nc.sync)
            load_st(1, 1, nc.scalar)
            nc.gpsimd.dma_start(scb[:], scb_d.ap())
            load_xbp(1, 0, nc.gpsimd)
            nc.scalar.dma_start(wt[1][:], wt_d.ap()[P:2 * P, :])
            nc.sync.dma_start(wt[2][:], wt_d.ap()[2 * P:3 * P, :])
            nc.scalar.dma_start(wt[3][:], wt_d.ap()[3 * P:4 * P, :])
            # stage 1 (block 1), then the big stage-2/3 slabs
            load_x8s(1, nc.gpsimd)
            load_xbp(0, 1, nc.sync)
            load_xbp(1, 1, nc.scalar)
            load_x8s(2, nc.gpsimd)
            load_xbp(0, 2, nc.sync)
            load_xbp(1, 2, nc.scalar)
            load_x8s(3, nc.gpsimd)
            load_xbp(0, 3, nc.sync)
            load_xbp(1, 3, nc.scalar)

            def stage_of(stages, n):
                for si, (b0, b1) in enumerate(stages):
                    if b0 <= n < b1:
                        return si, n - b0
                raise AssertionError(n)

            def rhs_for(u, n):
                if u < JP:
                    si, ln = stage_of(X8_STAGES, n)
                    if si == 0:
                        return x8s0[u][:].rearrange("p (i n) -> p i n", i=2)
                    base = ln * 2048 + u * 1024
                    return x8s[si][:, base:base + 1024].rearrange(
                        "p (i n) -> p i n", i=2)
                m = u - JP
                mp, mm = m // 2, m % 2
                si, ln = stage_of(XB_STAGES, n)
                base = ln * 1024 + mm * NB
                return xbp[mp][si][:, base:base + NB]

            yo_cur = [None] * OC

            def epilogue(n, c, ps):
                half = n % 2
                if half == 0:
                    yo_cur[c] = yo_pool.tile([P, 2 * NB], BF16, tag="yo",
                                             name=f"yo{n}_{c}")
                yo = yo_cur[c]
                dst = yo[:, half * NB:(half + 1) * NB]
                if c % 2 == 0:
                    nc.vector.tensor_scalar(dst, ps[:], scb[:, c:c + 1],
                                            scb[:, OC + c:OC + c + 1],
                                            Alu.mult, Alu.add)
                else:
                    nc.scalar.activation(dst, ps[:], Act.Identity,
                                         bias=scb[:, OC + c:OC + c + 1],
                                         scale=scb[:, c:c + 1])
                if n == NBC - 2:
                    # penultimate block: store its half immediately so it
                    # overlaps the last block's compute
                    nc.scalar.dma_start(
                        yt_d.ap()[c * P:(c + 1) * P, n * NB:(n + 1) * NB],
                        yo[:, 0:NB])
                elif n == NBC - 1:
                    # last block: per-c half stores fan out over the three
                    # DMA queues as each staggered epilogue completes ->
                    # short kernel tail
                    eng = (nc.sync, nc.scalar, nc.gpsimd)[c % 3]
                    eng.dma_start(
                        yt_d.ap()[c * P:(c + 1) * P, n * NB:(n + 1) * NB],
                        yo[:, NB:2 * NB])
                elif half == 1:
                    eng = nc.scalar if c % 2 == 1 else nc.sync
                    eng.dma_start(
                        yt_d.ap()[c * P:(c + 1) * P,
                                  (n - 1) * NB:(n + 1) * NB],
                        yo[:])

            def lhsT_dr(u, c):
                return st8[u][c // 4][:, :, (c % 4) * P:(c % 4 + 1) * P]

            # Per-bank unit order for blocks >=1: DoubleRow MMs at slots 0
            # and 3 so two DR MMs are never issued back-to-back (a DR pair
            # costs an extra ~30ns drain gap when adjacent).
            UORDER = (0, 2, 3, 1, 4, 5)

            def mm(s, c, n, ps):
                u = UORDER[s]
                if u < JP:
                    nc.tensor.matmul(ps[:], lhsT_dr(u, c),
                                     rhs_for(u, n), start=(s == 0), stop=False,
                                     perf_mode=DRMODE)
                else:
                    nc.tensor.matmul(ps[:], wt[u - JP][:, c * P:(c + 1) * P],
                                     rhs_for(u, n), start=(s == 0),
                                     stop=(s == NU - 1))

            # ---- block 0: phase A (DR c-sweeps, fp8 data only) then
            # phase B (bf16 units in skewed waves)
            yps = [ps_pool.tile([P, NB], F32, tag="ps", name=f"yp0_{c}")
                   for c in range(OC)]
            for u in range(JP):
                for c in range(OC):
                    nc.tensor.matmul(yps[c][:], lhsT_dr(u, c), rhs_for(u, 0),
                                     start=(u == 0), stop=False,
                                     perf_mode=DRMODE)
            for wv in range(KB + OC - 1):
                for c in range(OC):
                    s = wv - c
                    if 0 <= s < KB:
                        nc.tensor.matmul(
                            yps[c][:], wt[s][:, c * P:(c + 1) * P],
                            rhs_for(JP + s, 0), start=False,
                            stop=(s == KB - 1))
                        if s == KB - 1:
                            epilogue(0, c, yps[c])

            # ---- blocks 1..7: skewed waves.  MM(unit u, out-chunk c) at
            # wave u+c; bank completions stagger ~1 wave apart.
            for n in range(1, NBC):
                yps = [ps_pool.tile([P, NB], F32, tag="ps", name=f"yp{n}_{c}")
                       for c in range(OC)]
                for wv in range(NU + OC - 1):
                    for c in range(OC):
                        u = wv - c
                        if 0 <= u < NU:
                            mm(u, c, n, yps[c])
                            if u == NU - 1:
                                epilogue(n, c, yps[c])

    nc.compile()
    return nc


_NC = None


def _get_program():
    global _NC
    if _NC is None:
        _NC = build_program()
    return _NC


def kernel(x: np.ndarray, W: np.ndarray, b: np.ndarray) -> np.ndarray:
    assert x.shape == (BATCH, IN) and W.shape == (OUT, IN) and b.shape == (OUT,)
    nc = _get_program()

    Wf = np.asarray(W, dtype=np.float32)
    sgnT = np.where(Wf >= 0, np.float32(1.0), np.float32(-1.0)).T  # [in, out]
    # st cols are [oh (out half)][i (k subtile)][o'], so each out-half is
    # one contiguous 1KB-per-partition DMA
    st_pack = np.ascontiguousarray(
        sgnT[:K8].reshape(JP, 2, P, 2, OUT // 2).transpose(0, 2, 3, 1, 4)
        .reshape(JP * P, 2 * OUT)).astype(ml_dtypes.float8_e4m3)
    wt_pack = np.ascontiguousarray(sgnT[K8:]).astype(ml_dtypes.float8_e4m3)
    sc = np.maximum(np.abs(Wf).mean(axis=1), EPS).astype(np.float32)
    b32 = np.asarray(b, dtype=np.float32)
    # scb[p, c] = sc[c*128+p]; scb[p, OC+c] = b[c*128+p]
    scb = np.concatenate([sc.reshape(OC, P).T, b32.reshape(OC, P).T],
                         axis=1).astype(np.float32)
    scb = np.ascontiguousarray(scb)

    in_maps = []
    for c in range(N_CORES):
        xt = x[c * SHARD:(c + 1) * SHARD].T      # [in, n] view
        # x8 block-major: (j,i,p,nb,nn) -> (p, nb, j, i, nn)
        x8 = xt[:K8].astype(ml_dtypes.float8_e4m3)
        x8 = np.ascontiguousarray(
            x8.reshape(JP, 2, P, NBC, NB).transpose(2, 3, 0, 1, 4)
            .reshape(P, NBC * 2048))
        # xb: (mp,mm,p,nb,nn) -> (p, mp, nb, mm, nn)
        xb = xt[K8:].astype(ml_dtypes.bfloat16)
        xb = np.ascontiguousarray(
            xb.reshape(2, 2, P, NBC, NB).transpose(2, 0, 3, 1, 4)
            .reshape(P, 2 * NBC * 1024))
        in_maps.append({"x8": x8, "xb": xb, "st": st_pack, "wt": wt_pack,
                        "scb": scb})

    trace = bool(int(os.environ.get("BINLIN_TRACE", "0")))
    if trace:
        _install_trace_shim()
    res = run_bass_kernel_spmd(nc, in_maps, core_ids=list(range(N_CORES)),
                               trace=trace)
    if trace and res.exec_time_ns is not None:
        print(f"HW exec time: {res.exec_time_ns} ns", flush=True)

    y = np.empty((BATCH, OUT), dtype=np.float32)
    for c in range(N_CORES):
        y[c * SHARD:(c + 1) * SHARD] = res.results[c]["yt"].T.astype(np.float32)
    return y
